# revision 40
# baseline (speedup 1.0000x reference)
"""Block-diagonal MLP kernel for TRN2, 8 NeuronCores.

Computes out = x @ tanh(blocks * mask) where blocks is 4096x4096 with 16
diagonal 256x256 blocks (mask is the fixed block-diagonal pattern).
Off-diagonal entries of tanh(blocks*mask) are tanh(0)=0, so only the 16
diagonal blocks matter:

    out[:, 256k:256(k+1)] = x[:, 256k:256(k+1)] @ tanh(B_k)

Sharding: block-parallel. Core c owns blocks 2c and 2c+1 (512 contiguous
k/n-columns) and streams all 8192 rows of x transposed, computing

    outT_shard[n, m] = sum_k w[k, n] * xT_shard[k, m]

Precision: the wire is the bottleneck, so x ships as fp8 e3m4 (4 mantissa
bits): host sends x*2 in e3m4 and tanh(B)/2 in bf16 (power-of-2 scales
cancel exactly), halving the load stream vs bf16. Matmuls run e3m4
(moving) x bf16 (stationary) with fp32 PSUM accumulation; output returns
bf16 and is upcast on the host. End-to-end relative error 1.348e-2
(gate 2e-2), dominated by e3m4 x rounding.

Schedule highlights (from perfetto/NTFF iterations):
- Host repacks x into per-(group, block, m-half) tiles holding BOTH
  k-chunks a psum quad consumes, so one ring carries loads in exact
  consumption order; the first two tiles are split in half again so the
  first matmuls start as early as possible (the first ~6us of DMA run at
  ~200 B/ns while all 8 cores contend for HBM).
- All loads AND stores go on the Sync HWDGE ring in program order; the
  Scalar engine keeps its cycles for PSUM evacuation (store issues cost
  ~0.6us each on the issuing engine).
- kc is outer within a (ps_d, ps_a) psum quad so 4 consecutive matmuls
  share one stationary tile - the bf16-stationary/fp8-moving LDWEIGHTS
  otherwise costs ~43ns per matmul on top of the 216ns moving time.
- ps_d tiles are always evacuated by DVE and ps_a by ACT, from separate
  PSUM pools so each engine's buffer recycling alternates
  deterministically (a shared pool serialized the psum-reuse chain).
- A chain of dummy matmuls on a scratch tile warms the PE HAM clock gate
  (1.2 -> 2.4 GHz) during the DMA preamble, timed to end right when the
  first real matmul's data lands.
"""

import ml_dtypes
import numpy as np

import concourse.mybir as mybir
import concourse.tile as tile
from concourse import bacc
from concourse.bass_utils import run_bass_kernel_spmd

N_CORES = 8
N_ROWS = 8192            # rows of x / out
D = 4096                 # layer size
BLOCK = 256              # block size
BLOCKS_PER_CORE = 2      # 16 blocks / 8 cores
K_PER_CORE = BLOCKS_PER_CORE * BLOCK   # 512 k (and n) columns per core

M_GROUP = 4096           # m columns per group (row range of one out store)
N_GROUPS = N_ROWS // M_GROUP
MM_FREE = 512            # matmul moving free dim (one fp32 PSUM bank)
QUAD_M = 2048            # m columns per (ps_d, ps_a) psum quad
N_WARM = 8               # dummy matmuls to warm the PE clock gate
N_SPLIT = 2              # leading x tiles loaded as two halves

_nc_cache = None


def _build_nc():
    f32 = mybir.dt.float32
    bf16 = mybir.dt.bfloat16
    f8 = mybir.dt.float8e3

    nc = bacc.Bacc("TRN2")
    # x repacked on host: tile s = (g, blk, mh) holds both k-chunks for
    # m columns [g*4096 + mh*2048, +2048): [qe-chunk | qo-chunk], each
    # [128, 2048]. Tiles s < N_SPLIT are packed as interleaved halves
    # [qe h0 | qo h0 | qe h1 | qo h1] (1024 cols each) so they can load
    # as two 256 KiB DMAs.
    xP = nc.dram_tensor("xP", [8, 128, M_GROUP], f8, kind="ExternalInput")
    bsw = nc.dram_tensor("bsw", [128, 1024], bf16, kind="ExternalInput")
    outT = nc.dram_tensor("outT", [K_PER_CORE, N_ROWS], bf16,
                          kind="ExternalOutput")

    with tile.TileContext(nc) as tc:
        with (
            tc.tile_pool(name="wpool", bufs=1) as wpool,
            tc.tile_pool(name="bpool", bufs=1) as bpool,
            # 6 slots: s4-s7's loads each reuse a slot whose previous tile
            # is still being consumed, so the scheduler cannot hoist them
            # into the early HBM window (slot WAR = real flow control).
            tc.tile_pool(name="xpool", bufs=8) as xpool,
            tc.tile_pool(name="opool", bufs=8) as opool,
            tc.tile_pool(name="psd", bufs=2, space="PSUM") as psd_pool,
            tc.tile_pool(name="psa", bufs=2, space="PSUM") as psa_pool,
        ):
            # --- PE warm-up
            scr = wpool.tile([128, MM_FREE], bf16, name="warm_scr")
            nc.gpsimd.memset(scr[:], 0.5)
            ps_warm = psd_pool.tile([128, 2 * MM_FREE], f32, name="ps_warm",
                                    tag="psd")
            for _ in range(N_WARM):
                nc.tensor.matmul(
                    ps_warm[:, :MM_FREE], lhsT=scr[:, :128], rhs=scr[:],
                    start=True, stop=True,
                )

            # --- weights: blk0's half first (it gates the first matmul),
            # blk1's half later. Host has already tanh'd/swizzled/bf16'd.
            # column chunk (blk*2+kc)*2+ncol covers w[kc*128+p, ncol*128+j].
            b_mm = bpool.tile([128, 1024], bf16, name="b_mm")

            # --- x: tiles s0-s3 (plus both weight halves) up front on the
            # Sync ring in consumption order - they gate the start, so
            # nothing else may compete for the early (contended) HBM
            # window. s0 streams as four 128 KiB quarters [A0|B0|A1|B1]
            # (first matmuls start once w-half + ~0.25 MB land), s1 as two
            # halves. Tiles s4-s7 go on the Scalar ring, issued lazily
            # inside the compute loop so they neither steal early
            # bandwidth nor block ACT's evacuation stream.
            xs = {}

            def load_x(eng, s):
                if s < N_SPLIT:
                    for h in range(2):
                        t = xpool.tile([128, QUAD_M], f8, name=f"x{s}{h}",
                                       tag="xt")
                        eng.dma_start(
                            out=t[:],
                            in_=xP[s, :, h * QUAD_M:(h + 1) * QUAD_M],
                        )
                        xs[(s, h)] = t
                else:
                    t = xpool.tile([128, M_GROUP], f8, name=f"x{s}",
                                   tag="xt")
                    eng.dma_start(out=t[:], in_=xP[s])
                    xs[s] = t

            # s0's first half leads even the weight DMA: the first four
            # matmuls (ps_d of quad 0) need only s0h0 + blk0 weights, and
            # the x bytes are the bigger transfer.
            t = xpool.tile([128, QUAD_M], f8, name="x00", tag="xt")
            nc.sync.dma_start(out=t[:], in_=xP[0, :, :QUAD_M])
            xs[(0, 0)] = t
            nc.sync.dma_start(out=b_mm[:, :512], in_=bsw[:, :512])
            t = xpool.tile([128, QUAD_M], f8, name="x01", tag="xt")
            nc.sync.dma_start(out=t[:], in_=xP[0, :, QUAD_M:])
            xs[(0, 1)] = t
            load_x(nc.sync, 1)
            nc.sync.dma_start(out=b_mm[:, 512:], in_=bsw[:, 512:])
            load_x(nc.sync, 2)
            load_x(nc.sync, 3)
            # section index -> x tiles to issue on the Scalar ring right
            # before that section's matmuls are emitted
            lazy_loads = {1: [4], 2: [5], 3: [6], 4: [7]}

            def rhs(g, blk, kc, m):
                """rhs slice for m..m+512 of group g (m in [0, 4096))."""
                s = g * 4 + blk * 2 + m // QUAD_M
                off = m % QUAD_M
                if s < N_SPLIT:
                    t = xs[(s, off // 1024)]
                    return t[:, kc * 1024 + off % 1024:][:, :MM_FREE]
                return xs[s][:, kc * QUAD_M + off:][:, :MM_FREE]

            # --- matmul / evac / store pipeline
            section = 0
            for g in range(N_GROUPS):
                for blk in range(BLOCKS_PER_CORE):
                    for ncol in range(2):  # n chunk of 128 within the block
                        for s in lazy_loads.get(section, ()):
                            load_x(nc.scalar, s)
                        section += 1
                        out_sb = opool.tile([128, M_GROUP], bf16,
                                            name="out_sb")
                        for mh2 in range(M_GROUP // (4 * MM_FREE)):
                            ps_d = psd_pool.tile([128, 2 * MM_FREE], f32,
                                                 name="ps_d", tag="psd")
                            ps_a = psa_pool.tile([128, 2 * MM_FREE], f32,
                                                 name="ps_a")
                            base = 4 * mh2 * MM_FREE
                            for kc in range(2):
                                lcol = ((blk * 2 + kc) * 2 + ncol) * 128
                                for t, ps in ((0, ps_d), (1, ps_a)):
                                    for mi in range(2):
                                        m0 = base + (2 * t + mi) * MM_FREE
                                        nc.tensor.matmul(
                                            ps[:, mi * MM_FREE:
                                               (mi + 1) * MM_FREE],
                                            lhsT=b_mm[:, lcol:lcol + 128],
                                            rhs=rhs(g, blk, kc, m0),
                                            start=(kc == 0),
                                            stop=(kc == 1),
                                        )
                            nc.vector.tensor_copy(
                                out_sb[:, base:base + 2 * MM_FREE], ps_d[:]
                            )
                            nc.scalar.copy(
                                out_sb[:, base + 2 * MM_FREE:
                                       base + 4 * MM_FREE], ps_a[:]
                            )
                            r0 = blk * 256 + ncol * 128
                            c0 = g * M_GROUP + base
                            if section == 8 and mh2 == 1:
                                # final quad: store each half right after
                                # its own evac so the last bytes leave
                                # without waiting for both engines
                                nc.sync.dma_start(
                                    out=outT[r0:r0 + 128,
                                             c0:c0 + 2 * MM_FREE],
                                    in_=out_sb[:, base:base + 2 * MM_FREE],
                                )
                                nc.sync.dma_start(
                                    out=outT[r0:r0 + 128,
                                             c0 + 2 * MM_FREE:
                                             c0 + 4 * MM_FREE],
                                    in_=out_sb[:, base + 2 * MM_FREE:
                                               base + 4 * MM_FREE],
                                )
                            else:
                                nc.sync.dma_start(
                                    out=outT[r0:r0 + 128,
                                             c0:c0 + 4 * MM_FREE],
                                    in_=out_sb[:, base:base + 4 * MM_FREE],
                                )
    nc.compile()
    return nc


def _get_nc():
    global _nc_cache
    if _nc_cache is None:
        _nc_cache = _build_nc()
    return _nc_cache


def _make_in_maps(x, blocks):
    # x ships as e3m4 at 2x scale; weights absorb the 1/2 (both scales are
    # powers of two, so they are exact and cancel).
    xT = np.ascontiguousarray(x.T * np.float32(2.0)).astype(
        ml_dtypes.float8_e3m4
    )
    in_maps = []
    for c in range(N_CORES):
        k0 = c * K_PER_CORE
        xTc = xT[k0:k0 + K_PER_CORE]
        xp = np.empty((8, 128, M_GROUP), ml_dtypes.float8_e3m4)
        for g in range(N_GROUPS):
            for blk in range(BLOCKS_PER_CORE):
                for mh in range(2):
                    s = g * 4 + blk * 2 + mh
                    m0 = g * M_GROUP + mh * QUAD_M
                    A = xTc[2 * blk * 128:(2 * blk + 1) * 128, m0:m0 + QUAD_M]
                    B = xTc[(2 * blk + 1) * 128:(2 * blk + 2) * 128,
                            m0:m0 + QUAD_M]
                    if s < N_SPLIT:
                        xp[s, :, 0:1024] = A[:, :1024]
                        xp[s, :, 1024:2048] = B[:, :1024]
                        xp[s, :, 2048:3072] = A[:, 1024:]
                        xp[s, :, 3072:4096] = B[:, 1024:]
                    else:
                        xp[s, :, :QUAD_M] = A
                        xp[s, :, QUAD_M:] = B
        cols = []
        for blk in range(BLOCKS_PER_CORE):
            b0 = k0 + blk * BLOCK
            w = np.tanh(blocks[b0:b0 + BLOCK, b0:b0 + BLOCK]) * np.float32(0.5)
            # [kc*128+p, ncol*128+j] -> col = kc*256 + ncol*128 + j
            cols.append(
                w.reshape(2, 128, 2, 128).transpose(1, 0, 2, 3).reshape(128, 512)
            )
        bsw = np.concatenate(cols, axis=1).astype(ml_dtypes.bfloat16)
        in_maps.append({"xP": xp, "bsw": np.ascontiguousarray(bsw)})
    return in_maps


def _run(x, blocks, **spmd_kwargs):
    res = run_bass_kernel_spmd(
        _get_nc(), _make_in_maps(x, blocks), core_ids=list(range(N_CORES)),
        **spmd_kwargs,
    )
    out = np.empty((N_ROWS, D), np.float32)
    for c in range(N_CORES):
        shard = res.results[c]["outT"]
        out[:, c * K_PER_CORE:(c + 1) * K_PER_CORE] = shard.T.astype(np.float32)
    return out, res


def kernel(x, blocks, mask=None):
    out, _ = _run(np.asarray(x), np.asarray(blocks))
    return out


# revision 41
# speedup vs baseline: 1.0453x; 1.0453x over previous
"""Block-diagonal MLP kernel for TRN2, 8 NeuronCores.

Computes out = x @ tanh(blocks * mask) where blocks is 4096x4096 with 16
diagonal 256x256 blocks (mask is the fixed block-diagonal pattern).
Off-diagonal entries of tanh(blocks*mask) are tanh(0)=0, so only the 16
diagonal blocks matter:

    out[:, 256k:256(k+1)] = x[:, 256k:256(k+1)] @ tanh(B_k)

Sharding: block-parallel. Core c owns blocks 2c and 2c+1 (512 contiguous
k/n-columns) and streams all 8192 rows of x transposed, computing

    outT_shard[n, m] = sum_k w[k, n] * xT_shard[k, m]

Precision: the wire is the bottleneck, so x ships as fp8 e3m4 (4 mantissa
bits): host sends x*2 in e3m4 and tanh(B)/2 in bf16 (power-of-2 scales
cancel exactly), halving the load stream vs bf16. Matmuls run e3m4
(moving) x bf16 (stationary) with fp32 PSUM accumulation; output returns
bf16 and is upcast on the host. End-to-end relative error 1.348e-2
(gate 2e-2), dominated by e3m4 x rounding.

Schedule highlights (from perfetto/NTFF iterations):
- Host repacks x into per-(group, block, m-half) tiles holding BOTH
  k-chunks a psum quad consumes, so one ring carries loads in exact
  consumption order; the first two tiles are split in half again so the
  first matmuls start as early as possible (the first ~6us of DMA run at
  ~200 B/ns while all 8 cores contend for HBM).
- All loads AND stores go on the Sync HWDGE ring in program order; the
  Scalar engine keeps its cycles for PSUM evacuation (store issues cost
  ~0.6us each on the issuing engine).
- kc is outer within a (ps_d, ps_a) psum quad so 4 consecutive matmuls
  share one stationary tile - the bf16-stationary/fp8-moving LDWEIGHTS
  otherwise costs ~43ns per matmul on top of the 216ns moving time.
- ps_d tiles are always evacuated by DVE and ps_a by ACT, from separate
  PSUM pools so each engine's buffer recycling alternates
  deterministically (a shared pool serialized the psum-reuse chain).
- A chain of dummy matmuls on a scratch tile warms the PE HAM clock gate
  (1.2 -> 2.4 GHz) during the DMA preamble, timed to end right when the
  first real matmul's data lands.
"""

import ml_dtypes
import numpy as np

import concourse.mybir as mybir
import concourse.tile as tile
from concourse import bacc
from concourse.bass_utils import run_bass_kernel_spmd

N_CORES = 8
N_ROWS = 8192            # rows of x / out
D = 4096                 # layer size
BLOCK = 256              # block size
BLOCKS_PER_CORE = 2      # 16 blocks / 8 cores
K_PER_CORE = BLOCKS_PER_CORE * BLOCK   # 512 k (and n) columns per core

M_GROUP = 4096           # m columns per group (row range of one out store)
N_GROUPS = N_ROWS // M_GROUP
MM_FREE = 512            # matmul moving free dim (one fp32 PSUM bank)
QUAD_M = 2048            # m columns per (ps_d, ps_a) psum quad
N_WARM = 8               # dummy matmuls to warm the PE clock gate
N_SPLIT = 2              # leading x tiles loaded as two halves

_nc_cache = None


def _build_nc():
    f32 = mybir.dt.float32
    bf16 = mybir.dt.bfloat16
    f8 = mybir.dt.float8e3

    nc = bacc.Bacc("TRN2")
    # x repacked on host: tile s = (g, blk, mh) holds both k-chunks for
    # m columns [g*4096 + mh*2048, +2048): [qe-chunk | qo-chunk], each
    # [128, 2048]. Tiles s < N_SPLIT are packed as interleaved halves
    # [qe h0 | qo h0 | qe h1 | qo h1] (1024 cols each) so they can load
    # as two 256 KiB DMAs.
    xP = nc.dram_tensor("xP", [8, 128, M_GROUP], f8, kind="ExternalInput")
    bsw = nc.dram_tensor("bsw", [128, 1024], bf16, kind="ExternalInput")
    outT = nc.dram_tensor("outT", [K_PER_CORE, N_ROWS], bf16,
                          kind="ExternalOutput")

    with tile.TileContext(nc) as tc:
        with (
            tc.tile_pool(name="wpool", bufs=1) as wpool,
            tc.tile_pool(name="bpool", bufs=1) as bpool,
            tc.tile_pool(name="xpool", bufs=8) as xpool,
            tc.tile_pool(name="opool", bufs=8) as opool,
            tc.tile_pool(name="psd", bufs=2, space="PSUM") as psd_pool,
            tc.tile_pool(name="psa", bufs=2, space="PSUM") as psa_pool,
        ):
            # --- PE warm-up
            scr = wpool.tile([128, MM_FREE], bf16, name="warm_scr")
            nc.gpsimd.memset(scr[:], 0.5)
            ps_warm = psd_pool.tile([128, 2 * MM_FREE], f32, name="ps_warm",
                                    tag="psd")
            for _ in range(N_WARM):
                nc.tensor.matmul(
                    ps_warm[:, :MM_FREE], lhsT=scr[:, :128], rhs=scr[:],
                    start=True, stop=True,
                )

            # --- weights: blk0's half first (it gates the first matmul),
            # blk1's half later. Host has already tanh'd/swizzled/bf16'd.
            # column chunk (blk*2+kc)*2+ncol covers w[kc*128+p, ncol*128+j].
            b_mm = bpool.tile([128, 1024], bf16, name="b_mm")

            # --- x: tiles s0-s3 (plus both weight halves) up front on
            # the Sync ring in consumption order - they gate the start,
            # so nothing else may compete for the early (contended) HBM
            # window. s0/s1 stream as two 256 KiB halves each (smaller
            # pieces are DMA-issue-bound at ~0.65us per dma_start).
            # Tiles s4-s7 go on the Scalar ring, issued lazily inside the
            # compute loop (one section ahead of use) so they neither
            # steal early bandwidth nor block ACT's evacuation stream.
            xs = {}

            def load_x(eng, s):
                if s < N_SPLIT:
                    for h in range(2):
                        t = xpool.tile([128, QUAD_M], f8, name=f"x{s}{h}",
                                       tag="xt")
                        eng.dma_start(
                            out=t[:],
                            in_=xP[s, :, h * QUAD_M:(h + 1) * QUAD_M],
                        )
                        xs[(s, h)] = t
                else:
                    t = xpool.tile([128, M_GROUP], f8, name=f"x{s}",
                                   tag="xt")
                    eng.dma_start(out=t[:], in_=xP[s])
                    xs[s] = t

            # s0's first half leads even the weight DMA: the first four
            # matmuls (ps_d of quad 0) need only s0h0 + blk0 weights, and
            # the x bytes are the bigger transfer.
            t = xpool.tile([128, QUAD_M], f8, name="x00", tag="xt")
            nc.sync.dma_start(out=t[:], in_=xP[0, :, :QUAD_M])
            xs[(0, 0)] = t
            nc.sync.dma_start(out=b_mm[:, :512], in_=bsw[:, :512])
            t = xpool.tile([128, QUAD_M], f8, name="x01", tag="xt")
            nc.sync.dma_start(out=t[:], in_=xP[0, :, QUAD_M:])
            xs[(0, 1)] = t
            load_x(nc.sync, 1)
            nc.sync.dma_start(out=b_mm[:, 512:], in_=bsw[:, 512:])
            load_x(nc.sync, 2)
            load_x(nc.sync, 3)
            # section index -> x tiles to issue on the Scalar ring right
            # before that section's matmuls are emitted
            lazy_loads = {1: [4], 2: [5], 3: [6], 4: [7]}

            def rhs(g, blk, kc, m):
                """rhs slice for m..m+512 of group g (m in [0, 4096))."""
                s = g * 4 + blk * 2 + m // QUAD_M
                off = m % QUAD_M
                if s < N_SPLIT:
                    t = xs[(s, off // 1024)]
                    return t[:, kc * 1024 + off % 1024:][:, :MM_FREE]
                return xs[s][:, kc * QUAD_M + off:][:, :MM_FREE]

            # --- matmul / evac / store pipeline
            section = 0
            for g in range(N_GROUPS):
                for blk in range(BLOCKS_PER_CORE):
                    for ncol in range(2):  # n chunk of 128 within the block
                        for s in lazy_loads.get(section, ()):
                            load_x(nc.scalar, s)
                        section += 1
                        out_sb = opool.tile([128, M_GROUP], bf16,
                                            name="out_sb")
                        for mh2 in range(M_GROUP // (4 * MM_FREE)):
                            ps_d = psd_pool.tile([128, 2 * MM_FREE], f32,
                                                 name="ps_d", tag="psd")
                            ps_a = psa_pool.tile([128, 2 * MM_FREE], f32,
                                                 name="ps_a")
                            base = 4 * mh2 * MM_FREE
                            for kc in range(2):
                                lcol = ((blk * 2 + kc) * 2 + ncol) * 128
                                for t, ps in ((0, ps_d), (1, ps_a)):
                                    for mi in range(2):
                                        m0 = base + (2 * t + mi) * MM_FREE
                                        nc.tensor.matmul(
                                            ps[:, mi * MM_FREE:
                                               (mi + 1) * MM_FREE],
                                            lhsT=b_mm[:, lcol:lcol + 128],
                                            rhs=rhs(g, blk, kc, m0),
                                            start=(kc == 0),
                                            stop=(kc == 1),
                                        )
                            nc.vector.tensor_copy(
                                out_sb[:, base:base + 2 * MM_FREE], ps_d[:]
                            )
                            nc.scalar.copy(
                                out_sb[:, base + 2 * MM_FREE:
                                       base + 4 * MM_FREE], ps_a[:]
                            )
                            r0 = blk * 256 + ncol * 128
                            c0 = g * M_GROUP + base
                            if section == 8 and mh2 == 1:
                                # final quad: store each half right after
                                # its own evac so the last bytes leave
                                # without waiting for both engines
                                nc.sync.dma_start(
                                    out=outT[r0:r0 + 128,
                                             c0:c0 + 2 * MM_FREE],
                                    in_=out_sb[:, base:base + 2 * MM_FREE],
                                )
                                nc.sync.dma_start(
                                    out=outT[r0:r0 + 128,
                                             c0 + 2 * MM_FREE:
                                             c0 + 4 * MM_FREE],
                                    in_=out_sb[:, base + 2 * MM_FREE:
                                               base + 4 * MM_FREE],
                                )
                            else:
                                nc.sync.dma_start(
                                    out=outT[r0:r0 + 128,
                                             c0:c0 + 4 * MM_FREE],
                                    in_=out_sb[:, base:base + 4 * MM_FREE],
                                )
    nc.compile()
    return nc


def _get_nc():
    global _nc_cache
    if _nc_cache is None:
        _nc_cache = _build_nc()
    return _nc_cache


def _make_in_maps(x, blocks):
    # x ships as e3m4 at 2x scale; weights absorb the 1/2 (both scales are
    # powers of two, so they are exact and cancel).
    xT = np.ascontiguousarray(x.T * np.float32(2.0)).astype(
        ml_dtypes.float8_e3m4
    )
    in_maps = []
    for c in range(N_CORES):
        k0 = c * K_PER_CORE
        xTc = xT[k0:k0 + K_PER_CORE]
        xp = np.empty((8, 128, M_GROUP), ml_dtypes.float8_e3m4)
        for g in range(N_GROUPS):
            for blk in range(BLOCKS_PER_CORE):
                for mh in range(2):
                    s = g * 4 + blk * 2 + mh
                    m0 = g * M_GROUP + mh * QUAD_M
                    A = xTc[2 * blk * 128:(2 * blk + 1) * 128, m0:m0 + QUAD_M]
                    B = xTc[(2 * blk + 1) * 128:(2 * blk + 2) * 128,
                            m0:m0 + QUAD_M]
                    if s < N_SPLIT:
                        xp[s, :, 0:1024] = A[:, :1024]
                        xp[s, :, 1024:2048] = B[:, :1024]
                        xp[s, :, 2048:3072] = A[:, 1024:]
                        xp[s, :, 3072:4096] = B[:, 1024:]
                    else:
                        xp[s, :, :QUAD_M] = A
                        xp[s, :, QUAD_M:] = B
        cols = []
        for blk in range(BLOCKS_PER_CORE):
            b0 = k0 + blk * BLOCK
            w = np.tanh(blocks[b0:b0 + BLOCK, b0:b0 + BLOCK]) * np.float32(0.5)
            # [kc*128+p, ncol*128+j] -> col = kc*256 + ncol*128 + j
            cols.append(
                w.reshape(2, 128, 2, 128).transpose(1, 0, 2, 3).reshape(128, 512)
            )
        bsw = np.concatenate(cols, axis=1).astype(ml_dtypes.bfloat16)
        in_maps.append({"xP": xp, "bsw": np.ascontiguousarray(bsw)})
    return in_maps


def _run(x, blocks, **spmd_kwargs):
    res = run_bass_kernel_spmd(
        _get_nc(), _make_in_maps(x, blocks), core_ids=list(range(N_CORES)),
        **spmd_kwargs,
    )
    out = np.empty((N_ROWS, D), np.float32)
    for c in range(N_CORES):
        shard = res.results[c]["outT"]
        out[:, c * K_PER_CORE:(c + 1) * K_PER_CORE] = shard.T.astype(np.float32)
    return out, res


def kernel(x, blocks, mask=None):
    out, _ = _run(np.asarray(x), np.asarray(blocks))
    return out


# revision 43
# speedup vs baseline: 1.0516x; 1.0060x over previous
"""Block-diagonal MLP kernel for TRN2, 8 NeuronCores.

Computes out = x @ tanh(blocks * mask) where blocks is 4096x4096 with 16
diagonal 256x256 blocks (mask is the fixed block-diagonal pattern).
Off-diagonal entries of tanh(blocks*mask) are tanh(0)=0, so only the 16
diagonal blocks matter:

    out[:, 256k:256(k+1)] = x[:, 256k:256(k+1)] @ tanh(B_k)

Sharding: block-parallel. Core c owns blocks 2c and 2c+1 (512 contiguous
k/n-columns) and streams all 8192 rows of x transposed, computing

    outT_shard[n, m] = sum_k w[k, n] * xT_shard[k, m]

Precision: the wire is the bottleneck, so x ships as fp8 e3m4 (4 mantissa
bits): host sends x*2 in e3m4 and tanh(B)/2 in bf16 (power-of-2 scales
cancel exactly), halving the load stream vs bf16. Matmuls run e3m4
(moving) x bf16 (stationary) with fp32 PSUM accumulation; output returns
bf16 and is upcast on the host. End-to-end relative error 1.348e-2
(gate 2e-2), dominated by e3m4 x rounding.

Schedule highlights (from perfetto/NTFF iterations):
- Host repacks x into per-(group, block, m-half) tiles holding BOTH
  k-chunks a psum quad consumes, so one ring carries loads in exact
  consumption order; the first two tiles are split in half again so the
  first matmuls start as early as possible (the first ~6us of DMA run at
  ~200 B/ns while all 8 cores contend for HBM).
- All loads AND stores go on the Sync HWDGE ring in program order; the
  Scalar engine keeps its cycles for PSUM evacuation (store issues cost
  ~0.6us each on the issuing engine).
- kc is outer within a (ps_d, ps_a) psum quad so 4 consecutive matmuls
  share one stationary tile - the bf16-stationary/fp8-moving LDWEIGHTS
  otherwise costs ~43ns per matmul on top of the 216ns moving time.
- ps_d tiles are always evacuated by DVE and ps_a by ACT, from separate
  PSUM pools so each engine's buffer recycling alternates
  deterministically (a shared pool serialized the psum-reuse chain).
- A chain of dummy matmuls on a scratch tile warms the PE HAM clock gate
  (1.2 -> 2.4 GHz) during the DMA preamble, timed to end right when the
  first real matmul's data lands.
"""

import ml_dtypes
import numpy as np

import concourse.mybir as mybir
import concourse.tile as tile
from concourse import bacc
from concourse.bass_utils import run_bass_kernel_spmd

N_CORES = 8
N_ROWS = 8192            # rows of x / out
D = 4096                 # layer size
BLOCK = 256              # block size
BLOCKS_PER_CORE = 2      # 16 blocks / 8 cores
K_PER_CORE = BLOCKS_PER_CORE * BLOCK   # 512 k (and n) columns per core

M_GROUP = 4096           # m columns per group (row range of one out store)
N_GROUPS = N_ROWS // M_GROUP
MM_FREE = 512            # matmul moving free dim (one fp32 PSUM bank)
QUAD_M = 2048            # m columns per (ps_d, ps_a) psum quad
N_WARM = 8               # dummy matmuls to warm the PE clock gate
N_SPLIT = 2              # leading x tiles loaded as two halves

_nc_cache = None


def _build_nc():
    f32 = mybir.dt.float32
    bf16 = mybir.dt.bfloat16
    f8 = mybir.dt.float8e3

    nc = bacc.Bacc("TRN2")
    # x repacked on host: tile s = (g, blk, mh) holds both k-chunks for
    # m columns [g*4096 + mh*2048, +2048): [qe-chunk | qo-chunk], each
    # [128, 2048]. Tiles s < N_SPLIT are packed as interleaved halves
    # [qe h0 | qo h0 | qe h1 | qo h1] (1024 cols each) so they can load
    # as two 256 KiB DMAs.
    xP = nc.dram_tensor("xP", [8, 128, M_GROUP], f8, kind="ExternalInput")
    bsw = nc.dram_tensor("bsw", [128, 1024], bf16, kind="ExternalInput")
    outT = nc.dram_tensor("outT", [K_PER_CORE, N_ROWS], bf16,
                          kind="ExternalOutput")

    with tile.TileContext(nc) as tc:
        with (
            tc.tile_pool(name="wpool", bufs=1) as wpool,
            tc.tile_pool(name="bpool", bufs=1) as bpool,
            tc.tile_pool(name="xpool", bufs=8) as xpool,
            tc.tile_pool(name="opool", bufs=8) as opool,
            tc.tile_pool(name="psd", bufs=2, space="PSUM") as psd_pool,
            tc.tile_pool(name="psa", bufs=2, space="PSUM") as psa_pool,
        ):
            # --- PE warm-up
            scr = wpool.tile([128, MM_FREE], bf16, name="warm_scr")
            nc.gpsimd.memset(scr[:], 0.5)
            ps_warm = psd_pool.tile([128, 2 * MM_FREE], f32, name="ps_warm",
                                    tag="psd")
            for _ in range(N_WARM):
                nc.tensor.matmul(
                    ps_warm[:, :MM_FREE], lhsT=scr[:, :128], rhs=scr[:],
                    start=True, stop=True,
                )

            # --- weights: blk0's half first (it gates the first matmul),
            # blk1's half later. Host has already tanh'd/swizzled/bf16'd.
            # column chunk (blk*2+kc)*2+ncol covers w[kc*128+p, ncol*128+j].
            b_mm = bpool.tile([128, 1024], bf16, name="b_mm")

            # --- x: tiles s0-s3 (plus both weight halves) up front on
            # the Sync ring in consumption order - they gate the start,
            # so nothing else may compete for the early (contended) HBM
            # window. s0/s1 stream as two 256 KiB halves each (smaller
            # pieces are DMA-issue-bound at ~0.65us per dma_start).
            # Tiles s4-s7 go on the Scalar ring, issued lazily inside the
            # compute loop (one section ahead of use) so they neither
            # steal early bandwidth nor block ACT's evacuation stream.
            xs = {}

            def load_x(eng, s):
                if s < N_SPLIT:
                    for h in range(2):
                        t = xpool.tile([128, QUAD_M], f8, name=f"x{s}{h}",
                                       tag="xt")
                        eng.dma_start(
                            out=t[:],
                            in_=xP[s, :, h * QUAD_M:(h + 1) * QUAD_M],
                        )
                        xs[(s, h)] = t
                else:
                    t = xpool.tile([128, M_GROUP], f8, name=f"x{s}",
                                   tag="xt")
                    eng.dma_start(out=t[:], in_=xP[s])
                    xs[s] = t

            # head: the first two matmuls need only the A0 piece (q-even,
            # m 0-1023) plus blk0 weights, so stream s0h0 as its two
            # 128 KiB k-chunk pieces with the weight DMA in between -
            # the PE starts ~1.4us earlier and its activity extends the
            # warm-up chain for the HAM clock gate.
            xs0p = []
            for i, nm in enumerate(("x0A", "x0B")):
                t = xpool.tile([128, 1024], f8, name=nm, tag="xt")
                nc.sync.dma_start(out=t[:], in_=xP[0, :, i * 1024:
                                                   (i + 1) * 1024])
                xs0p.append(t)
                if i == 0:
                    nc.sync.dma_start(out=b_mm[:, :512], in_=bsw[:, :512])
            t = xpool.tile([128, QUAD_M], f8, name="x01", tag="xt")
            nc.sync.dma_start(out=t[:], in_=xP[0, :, QUAD_M:])
            xs[(0, 1)] = t
            load_x(nc.sync, 1)
            nc.sync.dma_start(out=b_mm[:, 512:], in_=bsw[:, 512:])
            load_x(nc.sync, 2)
            load_x(nc.sync, 3)
            # section index -> x tiles to issue on the Scalar ring right
            # before that section's matmuls are emitted
            lazy_loads = {1: [4], 2: [5], 3: [6], 4: [7]}

            def rhs(g, blk, kc, m):
                """rhs slice for m..m+512 of group g (m in [0, 4096))."""
                s = g * 4 + blk * 2 + m // QUAD_M
                off = m % QUAD_M
                if s == 0 and off < 1024:
                    return xs0p[kc][:, off:off + MM_FREE]
                if s < N_SPLIT:
                    t = xs[(s, off // 1024)]
                    return t[:, kc * 1024 + off % 1024:][:, :MM_FREE]
                return xs[s][:, kc * QUAD_M + off:][:, :MM_FREE]

            # --- matmul / evac / store pipeline
            section = 0
            for g in range(N_GROUPS):
                for blk in range(BLOCKS_PER_CORE):
                    for ncol in range(2):  # n chunk of 128 within the block
                        for s in lazy_loads.get(section, ()):
                            load_x(nc.scalar, s)
                        section += 1
                        out_sb = opool.tile([128, M_GROUP], bf16,
                                            name="out_sb")
                        for mh2 in range(M_GROUP // (4 * MM_FREE)):
                            ps_d = psd_pool.tile([128, 2 * MM_FREE], f32,
                                                 name="ps_d", tag="psd")
                            ps_a = psa_pool.tile([128, 2 * MM_FREE], f32,
                                                 name="ps_a")
                            base = 4 * mh2 * MM_FREE
                            for kc in range(2):
                                lcol = ((blk * 2 + kc) * 2 + ncol) * 128
                                for t, ps in ((0, ps_d), (1, ps_a)):
                                    for mi in range(2):
                                        m0 = base + (2 * t + mi) * MM_FREE
                                        nc.tensor.matmul(
                                            ps[:, mi * MM_FREE:
                                               (mi + 1) * MM_FREE],
                                            lhsT=b_mm[:, lcol:lcol + 128],
                                            rhs=rhs(g, blk, kc, m0),
                                            start=(kc == 0),
                                            stop=(kc == 1),
                                        )
                            nc.vector.tensor_copy(
                                out_sb[:, base:base + 2 * MM_FREE], ps_d[:]
                            )
                            nc.scalar.copy(
                                out_sb[:, base + 2 * MM_FREE:
                                       base + 4 * MM_FREE], ps_a[:]
                            )
                            r0 = blk * 256 + ncol * 128
                            c0 = g * M_GROUP + base
                            if section == 8 and mh2 == 1:
                                # final quad: store each half right after
                                # its own evac so the last bytes leave
                                # without waiting for both engines
                                nc.sync.dma_start(
                                    out=outT[r0:r0 + 128,
                                             c0:c0 + 2 * MM_FREE],
                                    in_=out_sb[:, base:base + 2 * MM_FREE],
                                )
                                nc.sync.dma_start(
                                    out=outT[r0:r0 + 128,
                                             c0 + 2 * MM_FREE:
                                             c0 + 4 * MM_FREE],
                                    in_=out_sb[:, base + 2 * MM_FREE:
                                               base + 4 * MM_FREE],
                                )
                            else:
                                nc.sync.dma_start(
                                    out=outT[r0:r0 + 128,
                                             c0:c0 + 4 * MM_FREE],
                                    in_=out_sb[:, base:base + 4 * MM_FREE],
                                )
    nc.compile()
    return nc


def _get_nc():
    global _nc_cache
    if _nc_cache is None:
        _nc_cache = _build_nc()
    return _nc_cache


def _make_in_maps(x, blocks):
    # x ships as e3m4 at 2x scale; weights absorb the 1/2 (both scales are
    # powers of two, so they are exact and cancel).
    xT = np.ascontiguousarray(x.T * np.float32(2.0)).astype(
        ml_dtypes.float8_e3m4
    )
    in_maps = []
    for c in range(N_CORES):
        k0 = c * K_PER_CORE
        xTc = xT[k0:k0 + K_PER_CORE]
        xp = np.empty((8, 128, M_GROUP), ml_dtypes.float8_e3m4)
        for g in range(N_GROUPS):
            for blk in range(BLOCKS_PER_CORE):
                for mh in range(2):
                    s = g * 4 + blk * 2 + mh
                    m0 = g * M_GROUP + mh * QUAD_M
                    A = xTc[2 * blk * 128:(2 * blk + 1) * 128, m0:m0 + QUAD_M]
                    B = xTc[(2 * blk + 1) * 128:(2 * blk + 2) * 128,
                            m0:m0 + QUAD_M]
                    if s < N_SPLIT:
                        xp[s, :, 0:1024] = A[:, :1024]
                        xp[s, :, 1024:2048] = B[:, :1024]
                        xp[s, :, 2048:3072] = A[:, 1024:]
                        xp[s, :, 3072:4096] = B[:, 1024:]
                    else:
                        xp[s, :, :QUAD_M] = A
                        xp[s, :, QUAD_M:] = B
        cols = []
        for blk in range(BLOCKS_PER_CORE):
            b0 = k0 + blk * BLOCK
            w = np.tanh(blocks[b0:b0 + BLOCK, b0:b0 + BLOCK]) * np.float32(0.5)
            # [kc*128+p, ncol*128+j] -> col = kc*256 + ncol*128 + j
            cols.append(
                w.reshape(2, 128, 2, 128).transpose(1, 0, 2, 3).reshape(128, 512)
            )
        bsw = np.concatenate(cols, axis=1).astype(ml_dtypes.bfloat16)
        in_maps.append({"xP": xp, "bsw": np.ascontiguousarray(bsw)})
    return in_maps


def _run(x, blocks, **spmd_kwargs):
    res = run_bass_kernel_spmd(
        _get_nc(), _make_in_maps(x, blocks), core_ids=list(range(N_CORES)),
        **spmd_kwargs,
    )
    out = np.empty((N_ROWS, D), np.float32)
    for c in range(N_CORES):
        shard = res.results[c]["outT"]
        out[:, c * K_PER_CORE:(c + 1) * K_PER_CORE] = shard.T.astype(np.float32)
    return out, res


def kernel(x, blocks, mask=None):
    out, _ = _run(np.asarray(x), np.asarray(blocks))
    return out


# revision 44
# speedup vs baseline: 1.0836x; 1.0304x over previous
"""Block-diagonal MLP kernel for TRN2, 8 NeuronCores.

Computes out = x @ tanh(blocks * mask) where blocks is 4096x4096 with 16
diagonal 256x256 blocks (mask is the fixed block-diagonal pattern).
Off-diagonal entries of tanh(blocks*mask) are tanh(0)=0, so only the 16
diagonal blocks matter:

    out[:, 256k:256(k+1)] = x[:, 256k:256(k+1)] @ tanh(B_k)

Sharding: block-parallel. Core c owns blocks 2c and 2c+1 (512 contiguous
k/n-columns) and streams all 8192 rows of x transposed, computing

    outT_shard[n, m] = sum_k w[k, n] * xT_shard[k, m]

Precision: the wire is the bottleneck, so x ships as fp8 e3m4 (4 mantissa
bits): host sends x*2 in e3m4 and tanh(B)/2 in bf16 (power-of-2 scales
cancel exactly), halving the load stream vs bf16. Matmuls run e3m4
(moving) x bf16 (stationary) with fp32 PSUM accumulation; output returns
bf16 and is upcast on the host. End-to-end relative error 1.348e-2
(gate 2e-2), dominated by e3m4 x rounding.

Schedule highlights (from perfetto/NTFF iterations):
- Host repacks x into per-(group, block, m-half) tiles holding BOTH
  k-chunks a psum quad consumes, so one ring carries loads in exact
  consumption order; the first two tiles are split in half again so the
  first matmuls start as early as possible (the first ~6us of DMA run at
  ~200 B/ns while all 8 cores contend for HBM).
- All loads AND stores go on the Sync HWDGE ring in program order; the
  Scalar engine keeps its cycles for PSUM evacuation (store issues cost
  ~0.6us each on the issuing engine).
- kc is outer within a (ps_d, ps_a) psum quad so 4 consecutive matmuls
  share one stationary tile - the bf16-stationary/fp8-moving LDWEIGHTS
  otherwise costs ~43ns per matmul on top of the 216ns moving time.
- ps_d tiles are always evacuated by DVE and ps_a by ACT, from separate
  PSUM pools so each engine's buffer recycling alternates
  deterministically (a shared pool serialized the psum-reuse chain).
- A chain of dummy matmuls on a scratch tile warms the PE HAM clock gate
  (1.2 -> 2.4 GHz) during the DMA preamble, timed to end right when the
  first real matmul's data lands.
"""

import ml_dtypes
import numpy as np

import concourse.mybir as mybir
import concourse.tile as tile
from concourse import bacc
from concourse.bass_utils import run_bass_kernel_spmd

N_CORES = 8
N_ROWS = 8192            # rows of x / out
D = 4096                 # layer size
BLOCK = 256              # block size
BLOCKS_PER_CORE = 2      # 16 blocks / 8 cores
K_PER_CORE = BLOCKS_PER_CORE * BLOCK   # 512 k (and n) columns per core

M_GROUP = 4096           # m columns per group (row range of one out store)
N_GROUPS = N_ROWS // M_GROUP
MM_FREE = 512            # matmul moving free dim (one fp32 PSUM bank)
QUAD_M = 2048            # m columns per (ps_d, ps_a) psum quad
N_WARM = 5               # dummy matmuls to warm the PE clock gate
N_SPLIT = 2              # leading x tiles loaded as two halves

_nc_cache = None


def _build_nc():
    f32 = mybir.dt.float32
    bf16 = mybir.dt.bfloat16
    f8 = mybir.dt.float8e3

    nc = bacc.Bacc("TRN2")
    # x repacked on host: tile s = (g, blk, mh) holds both k-chunks for
    # m columns [g*4096 + mh*2048, +2048): [qe-chunk | qo-chunk], each
    # [128, 2048]. Tiles s < N_SPLIT are packed as interleaved halves
    # [qe h0 | qo h0 | qe h1 | qo h1] (1024 cols each) so they can load
    # as two 256 KiB DMAs.
    xP = nc.dram_tensor("xP", [8, 128, M_GROUP], f8, kind="ExternalInput")
    bsw = nc.dram_tensor("bsw", [128, 1024], bf16, kind="ExternalInput")
    outT = nc.dram_tensor("outT", [K_PER_CORE, N_ROWS], bf16,
                          kind="ExternalOutput")

    with tile.TileContext(nc) as tc:
        with (
            tc.tile_pool(name="wpool", bufs=1) as wpool,
            tc.tile_pool(name="bpool", bufs=1) as bpool,
            tc.tile_pool(name="xpool", bufs=8) as xpool,
            tc.tile_pool(name="opool", bufs=8) as opool,
            tc.tile_pool(name="psd", bufs=2, space="PSUM") as psd_pool,
            tc.tile_pool(name="psa", bufs=2, space="PSUM") as psa_pool,
        ):
            # --- PE warm-up
            scr = wpool.tile([128, MM_FREE], bf16, name="warm_scr")
            nc.gpsimd.memset(scr[:], 0.5)
            ps_warm = psd_pool.tile([128, 2 * MM_FREE], f32, name="ps_warm",
                                    tag="psd")
            for _ in range(N_WARM):
                nc.tensor.matmul(
                    ps_warm[:, :MM_FREE], lhsT=scr[:, :128], rhs=scr[:],
                    start=True, stop=True,
                )

            # --- weights: blk0's half first (it gates the first matmul),
            # blk1's half later. Host has already tanh'd/swizzled/bf16'd.
            # column chunk (blk*2+kc)*2+ncol covers w[kc*128+p, ncol*128+j].
            b_mm = bpool.tile([128, 1024], bf16, name="b_mm")

            # --- x: tiles s0-s3 (plus both weight halves) up front on
            # the Sync ring in consumption order - they gate the start,
            # so nothing else may compete for the early (contended) HBM
            # window. s0/s1 stream as two 256 KiB halves each (smaller
            # pieces are DMA-issue-bound at ~0.65us per dma_start).
            # Tiles s4-s7 go on the Scalar ring, issued lazily inside the
            # compute loop (one section ahead of use) so they neither
            # steal early bandwidth nor block ACT's evacuation stream.
            xs = {}

            def load_x(eng, s):
                if s < N_SPLIT:
                    for h in range(2):
                        t = xpool.tile([128, QUAD_M], f8, name=f"x{s}{h}",
                                       tag="xt")
                        eng.dma_start(
                            out=t[:],
                            in_=xP[s, :, h * QUAD_M:(h + 1) * QUAD_M],
                        )
                        xs[(s, h)] = t
                else:
                    t = xpool.tile([128, M_GROUP], f8, name=f"x{s}",
                                   tag="xt")
                    eng.dma_start(out=t[:], in_=xP[s])
                    xs[s] = t

            # head: the first two matmuls need only the A0 piece (q-even,
            # m 0-1023) plus blk0 weights, so stream s0h0 as its two
            # 128 KiB k-chunk pieces with the weight DMA in between -
            # the PE starts ~1.4us earlier and its activity extends the
            # warm-up chain for the HAM clock gate.
            xs0p = []
            for i, nm in enumerate(("x0A", "x0B")):
                t = xpool.tile([128, 1024], f8, name=nm, tag="xt")
                nc.sync.dma_start(out=t[:], in_=xP[0, :, i * 1024:
                                                   (i + 1) * 1024])
                xs0p.append(t)
                if i == 0:
                    nc.sync.dma_start(out=b_mm[:, :512], in_=bsw[:, :512])
            t = xpool.tile([128, QUAD_M], f8, name="x01", tag="xt")
            nc.sync.dma_start(out=t[:], in_=xP[0, :, QUAD_M:])
            xs[(0, 1)] = t
            load_x(nc.sync, 1)
            nc.sync.dma_start(out=b_mm[:, 512:], in_=bsw[:, 512:])
            load_x(nc.sync, 2)
            load_x(nc.sync, 3)
            # section index -> x tiles to issue on the Scalar ring right
            # before that section's matmuls are emitted
            lazy_loads = {1: [4], 2: [5], 3: [6], 4: [7]}

            def rhs(g, blk, kc, m):
                """rhs slice for m..m+512 of group g (m in [0, 4096))."""
                s = g * 4 + blk * 2 + m // QUAD_M
                off = m % QUAD_M
                if s == 0 and off < 1024:
                    return xs0p[kc][:, off:off + MM_FREE]
                if s < N_SPLIT:
                    t = xs[(s, off // 1024)]
                    return t[:, kc * 1024 + off % 1024:][:, :MM_FREE]
                return xs[s][:, kc * QUAD_M + off:][:, :MM_FREE]

            # --- matmul / evac / store pipeline
            section = 0
            for g in range(N_GROUPS):
                for blk in range(BLOCKS_PER_CORE):
                    for ncol in range(2):  # n chunk of 128 within the block
                        for s in lazy_loads.get(section, ()):
                            load_x(nc.scalar, s)
                        section += 1
                        out_sb = opool.tile([128, M_GROUP], bf16,
                                            name="out_sb")
                        for mh2 in range(M_GROUP // (4 * MM_FREE)):
                            ps_d = psd_pool.tile([128, 2 * MM_FREE], f32,
                                                 name="ps_d", tag="psd")
                            ps_a = psa_pool.tile([128, 2 * MM_FREE], f32,
                                                 name="ps_a")
                            base = 4 * mh2 * MM_FREE
                            for kc in range(2):
                                lcol = ((blk * 2 + kc) * 2 + ncol) * 128
                                for t, ps in ((0, ps_d), (1, ps_a)):
                                    for mi in range(2):
                                        m0 = base + (2 * t + mi) * MM_FREE
                                        nc.tensor.matmul(
                                            ps[:, mi * MM_FREE:
                                               (mi + 1) * MM_FREE],
                                            lhsT=b_mm[:, lcol:lcol + 128],
                                            rhs=rhs(g, blk, kc, m0),
                                            start=(kc == 0),
                                            stop=(kc == 1),
                                        )
                            nc.vector.tensor_copy(
                                out_sb[:, base:base + 2 * MM_FREE], ps_d[:]
                            )
                            nc.scalar.copy(
                                out_sb[:, base + 2 * MM_FREE:
                                       base + 4 * MM_FREE], ps_a[:]
                            )
                            r0 = blk * 256 + ncol * 128
                            c0 = g * M_GROUP + base
                            if section == 8 and mh2 == 1:
                                # final quad: store each half right after
                                # its own evac so the last bytes leave
                                # without waiting for both engines
                                nc.sync.dma_start(
                                    out=outT[r0:r0 + 128,
                                             c0:c0 + 2 * MM_FREE],
                                    in_=out_sb[:, base:base + 2 * MM_FREE],
                                )
                                nc.sync.dma_start(
                                    out=outT[r0:r0 + 128,
                                             c0 + 2 * MM_FREE:
                                             c0 + 4 * MM_FREE],
                                    in_=out_sb[:, base + 2 * MM_FREE:
                                               base + 4 * MM_FREE],
                                )
                            else:
                                nc.sync.dma_start(
                                    out=outT[r0:r0 + 128,
                                             c0:c0 + 4 * MM_FREE],
                                    in_=out_sb[:, base:base + 4 * MM_FREE],
                                )
    nc.compile()
    return nc


def _get_nc():
    global _nc_cache
    if _nc_cache is None:
        _nc_cache = _build_nc()
    return _nc_cache


def _make_in_maps(x, blocks):
    # x ships as e3m4 at 2x scale; weights absorb the 1/2 (both scales are
    # powers of two, so they are exact and cancel).
    xT = np.ascontiguousarray(x.T * np.float32(2.0)).astype(
        ml_dtypes.float8_e3m4
    )
    in_maps = []
    for c in range(N_CORES):
        k0 = c * K_PER_CORE
        xTc = xT[k0:k0 + K_PER_CORE]
        xp = np.empty((8, 128, M_GROUP), ml_dtypes.float8_e3m4)
        for g in range(N_GROUPS):
            for blk in range(BLOCKS_PER_CORE):
                for mh in range(2):
                    s = g * 4 + blk * 2 + mh
                    m0 = g * M_GROUP + mh * QUAD_M
                    A = xTc[2 * blk * 128:(2 * blk + 1) * 128, m0:m0 + QUAD_M]
                    B = xTc[(2 * blk + 1) * 128:(2 * blk + 2) * 128,
                            m0:m0 + QUAD_M]
                    if s < N_SPLIT:
                        xp[s, :, 0:1024] = A[:, :1024]
                        xp[s, :, 1024:2048] = B[:, :1024]
                        xp[s, :, 2048:3072] = A[:, 1024:]
                        xp[s, :, 3072:4096] = B[:, 1024:]
                    else:
                        xp[s, :, :QUAD_M] = A
                        xp[s, :, QUAD_M:] = B
        cols = []
        for blk in range(BLOCKS_PER_CORE):
            b0 = k0 + blk * BLOCK
            w = np.tanh(blocks[b0:b0 + BLOCK, b0:b0 + BLOCK]) * np.float32(0.5)
            # [kc*128+p, ncol*128+j] -> col = kc*256 + ncol*128 + j
            cols.append(
                w.reshape(2, 128, 2, 128).transpose(1, 0, 2, 3).reshape(128, 512)
            )
        bsw = np.concatenate(cols, axis=1).astype(ml_dtypes.bfloat16)
        in_maps.append({"xP": xp, "bsw": np.ascontiguousarray(bsw)})
    return in_maps


def _run(x, blocks, **spmd_kwargs):
    res = run_bass_kernel_spmd(
        _get_nc(), _make_in_maps(x, blocks), core_ids=list(range(N_CORES)),
        **spmd_kwargs,
    )
    out = np.empty((N_ROWS, D), np.float32)
    for c in range(N_CORES):
        shard = res.results[c]["outT"]
        out[:, c * K_PER_CORE:(c + 1) * K_PER_CORE] = shard.T.astype(np.float32)
    return out, res


def kernel(x, blocks, mask=None):
    out, _ = _run(np.asarray(x), np.asarray(blocks))
    return out


# revision 45
# speedup vs baseline: 1.1058x; 1.0204x over previous
"""Block-diagonal MLP kernel for TRN2, 8 NeuronCores.

Computes out = x @ tanh(blocks * mask) where blocks is 4096x4096 with 16
diagonal 256x256 blocks (mask is the fixed block-diagonal pattern).
Off-diagonal entries of tanh(blocks*mask) are tanh(0)=0, so only the 16
diagonal blocks matter:

    out[:, 256k:256(k+1)] = x[:, 256k:256(k+1)] @ tanh(B_k)

Sharding: block-parallel. Core c owns blocks 2c and 2c+1 (512 contiguous
k/n-columns) and streams all 8192 rows of x transposed, computing

    outT_shard[n, m] = sum_k w[k, n] * xT_shard[k, m]

Precision: the wire is the bottleneck, so x ships as fp8 e3m4 (4 mantissa
bits): host sends x*2 in e3m4 and tanh(B)/2 in bf16 (power-of-2 scales
cancel exactly), halving the load stream vs bf16. Matmuls run e3m4
(moving) x bf16 (stationary) with fp32 PSUM accumulation; output returns
bf16 and is upcast on the host. End-to-end relative error 1.348e-2
(gate 2e-2), dominated by e3m4 x rounding.

Schedule highlights (from perfetto/NTFF iterations):
- Host repacks x into per-(group, block, m-half) tiles holding BOTH
  k-chunks a psum quad consumes, so one ring carries loads in exact
  consumption order; the first two tiles are split in half again so the
  first matmuls start as early as possible (the first ~6us of DMA run at
  ~200 B/ns while all 8 cores contend for HBM).
- All loads AND stores go on the Sync HWDGE ring in program order; the
  Scalar engine keeps its cycles for PSUM evacuation (store issues cost
  ~0.6us each on the issuing engine).
- kc is outer within a (ps_d, ps_a) psum quad so 4 consecutive matmuls
  share one stationary tile - the bf16-stationary/fp8-moving LDWEIGHTS
  otherwise costs ~43ns per matmul on top of the 216ns moving time.
- ps_d tiles are always evacuated by DVE and ps_a by ACT, from separate
  PSUM pools so each engine's buffer recycling alternates
  deterministically (a shared pool serialized the psum-reuse chain).
- A chain of dummy matmuls on a scratch tile warms the PE HAM clock gate
  (1.2 -> 2.4 GHz) during the DMA preamble, timed to end right when the
  first real matmul's data lands.
"""

import ml_dtypes
import numpy as np

import concourse.mybir as mybir
import concourse.tile as tile
from concourse import bacc
from concourse.bass_utils import run_bass_kernel_spmd

N_CORES = 8
N_ROWS = 8192            # rows of x / out
D = 4096                 # layer size
BLOCK = 256              # block size
BLOCKS_PER_CORE = 2      # 16 blocks / 8 cores
K_PER_CORE = BLOCKS_PER_CORE * BLOCK   # 512 k (and n) columns per core

M_GROUP = 4096           # m columns per group (row range of one out store)
N_GROUPS = N_ROWS // M_GROUP
MM_FREE = 512            # matmul moving free dim (one fp32 PSUM bank)
QUAD_M = 2048            # m columns per (ps_d, ps_a) psum quad
N_WARM = 5               # dummy matmuls to warm the PE clock gate
N_SPLIT = 1              # x tiles below this index load in halves (s0
                         # additionally leads with two 128 KiB pieces)

_nc_cache = None


def _build_nc():
    f32 = mybir.dt.float32
    bf16 = mybir.dt.bfloat16
    f8 = mybir.dt.float8e3

    nc = bacc.Bacc("TRN2")
    # x repacked on host: tile s = (g, blk, mh) holds both k-chunks for
    # m columns [g*4096 + mh*2048, +2048): [qe-chunk | qo-chunk], each
    # [128, 2048]. Tiles s < N_SPLIT are packed as interleaved halves
    # [qe h0 | qo h0 | qe h1 | qo h1] (1024 cols each) so they can load
    # as two 256 KiB DMAs.
    xP = nc.dram_tensor("xP", [8, 128, M_GROUP], f8, kind="ExternalInput")
    bsw = nc.dram_tensor("bsw", [128, 1024], bf16, kind="ExternalInput")
    outT = nc.dram_tensor("outT", [K_PER_CORE, N_ROWS], bf16,
                          kind="ExternalOutput")

    with tile.TileContext(nc) as tc:
        with (
            tc.tile_pool(name="wpool", bufs=1) as wpool,
            tc.tile_pool(name="bpool", bufs=1) as bpool,
            tc.tile_pool(name="xpool", bufs=8) as xpool,
            tc.tile_pool(name="opool", bufs=8) as opool,
            tc.tile_pool(name="psd", bufs=2, space="PSUM") as psd_pool,
            tc.tile_pool(name="psa", bufs=2, space="PSUM") as psa_pool,
        ):
            # --- PE warm-up
            scr = wpool.tile([128, MM_FREE], bf16, name="warm_scr")
            nc.gpsimd.memset(scr[:], 0.5)
            ps_warm = psd_pool.tile([128, 2 * MM_FREE], f32, name="ps_warm",
                                    tag="psd")
            for _ in range(N_WARM):
                nc.tensor.matmul(
                    ps_warm[:, :MM_FREE], lhsT=scr[:, :128], rhs=scr[:],
                    start=True, stop=True,
                )

            # --- weights: blk0's half first (it gates the first matmul),
            # blk1's half later. Host has already tanh'd/swizzled/bf16'd.
            # column chunk (blk*2+kc)*2+ncol covers w[kc*128+p, ncol*128+j].
            b_mm = bpool.tile([128, 1024], bf16, name="b_mm")

            # --- x: tiles s0-s3 (plus both weight halves) up front on
            # the Sync ring in consumption order - they gate the start,
            # so nothing else may compete for the early (contended) HBM
            # window. s0/s1 stream as two 256 KiB halves each (smaller
            # pieces are DMA-issue-bound at ~0.65us per dma_start).
            # Tiles s4-s7 go on the Scalar ring, issued lazily inside the
            # compute loop (one section ahead of use) so they neither
            # steal early bandwidth nor block ACT's evacuation stream.
            xs = {}

            def load_x(eng, s):
                if s < N_SPLIT:
                    for h in range(2):
                        t = xpool.tile([128, QUAD_M], f8, name=f"x{s}{h}",
                                       tag="xt")
                        eng.dma_start(
                            out=t[:],
                            in_=xP[s, :, h * QUAD_M:(h + 1) * QUAD_M],
                        )
                        xs[(s, h)] = t
                else:
                    t = xpool.tile([128, M_GROUP], f8, name=f"x{s}",
                                   tag="xt")
                    eng.dma_start(out=t[:], in_=xP[s])
                    xs[s] = t

            # head: the first two matmuls need only the A0 piece (q-even,
            # m 0-1023) plus blk0 weights, so stream s0h0 as its two
            # 128 KiB k-chunk pieces with the weight DMA in between -
            # the PE starts ~1.4us earlier and its activity extends the
            # warm-up chain for the HAM clock gate.
            xs0p = []
            for i, nm in enumerate(("x0A", "x0B")):
                t = xpool.tile([128, 1024], f8, name=nm, tag="xt")
                nc.sync.dma_start(out=t[:], in_=xP[0, :, i * 1024:
                                                   (i + 1) * 1024])
                xs0p.append(t)
                if i == 0:
                    nc.sync.dma_start(out=b_mm[:, :512], in_=bsw[:, :512])
            t = xpool.tile([128, QUAD_M], f8, name="x01", tag="xt")
            nc.sync.dma_start(out=t[:], in_=xP[0, :, QUAD_M:])
            xs[(0, 1)] = t
            load_x(nc.sync, 1)
            nc.sync.dma_start(out=b_mm[:, 512:], in_=bsw[:, 512:])
            load_x(nc.sync, 2)
            load_x(nc.sync, 3)
            # section index -> x tiles to issue on the Scalar ring right
            # before that section's matmuls are emitted
            lazy_loads = {1: [4], 2: [5], 3: [6], 4: [7]}

            def rhs(g, blk, kc, m):
                """rhs slice for m..m+512 of group g (m in [0, 4096))."""
                s = g * 4 + blk * 2 + m // QUAD_M
                off = m % QUAD_M
                if s == 0 and off < 1024:
                    return xs0p[kc][:, off:off + MM_FREE]
                if s < N_SPLIT:
                    t = xs[(s, off // 1024)]
                    return t[:, kc * 1024 + off % 1024:][:, :MM_FREE]
                return xs[s][:, kc * QUAD_M + off:][:, :MM_FREE]

            # --- matmul / evac / store pipeline
            section = 0
            for g in range(N_GROUPS):
                for blk in range(BLOCKS_PER_CORE):
                    for ncol in range(2):  # n chunk of 128 within the block
                        for s in lazy_loads.get(section, ()):
                            load_x(nc.scalar, s)
                        section += 1
                        out_sb = opool.tile([128, M_GROUP], bf16,
                                            name="out_sb")
                        for mh2 in range(M_GROUP // (4 * MM_FREE)):
                            ps_d = psd_pool.tile([128, 2 * MM_FREE], f32,
                                                 name="ps_d", tag="psd")
                            ps_a = psa_pool.tile([128, 2 * MM_FREE], f32,
                                                 name="ps_a")
                            base = 4 * mh2 * MM_FREE
                            for kc in range(2):
                                lcol = ((blk * 2 + kc) * 2 + ncol) * 128
                                for t, ps in ((0, ps_d), (1, ps_a)):
                                    for mi in range(2):
                                        m0 = base + (2 * t + mi) * MM_FREE
                                        nc.tensor.matmul(
                                            ps[:, mi * MM_FREE:
                                               (mi + 1) * MM_FREE],
                                            lhsT=b_mm[:, lcol:lcol + 128],
                                            rhs=rhs(g, blk, kc, m0),
                                            start=(kc == 0),
                                            stop=(kc == 1),
                                        )
                            nc.vector.tensor_copy(
                                out_sb[:, base:base + 2 * MM_FREE], ps_d[:]
                            )
                            nc.scalar.copy(
                                out_sb[:, base + 2 * MM_FREE:
                                       base + 4 * MM_FREE], ps_a[:]
                            )
                            r0 = blk * 256 + ncol * 128
                            c0 = g * M_GROUP + base
                            if section == 8 and mh2 == 1:
                                # final quad: store each half right after
                                # its own evac so the last bytes leave
                                # without waiting for both engines
                                nc.sync.dma_start(
                                    out=outT[r0:r0 + 128,
                                             c0:c0 + 2 * MM_FREE],
                                    in_=out_sb[:, base:base + 2 * MM_FREE],
                                )
                                nc.sync.dma_start(
                                    out=outT[r0:r0 + 128,
                                             c0 + 2 * MM_FREE:
                                             c0 + 4 * MM_FREE],
                                    in_=out_sb[:, base + 2 * MM_FREE:
                                               base + 4 * MM_FREE],
                                )
                            else:
                                nc.sync.dma_start(
                                    out=outT[r0:r0 + 128,
                                             c0:c0 + 4 * MM_FREE],
                                    in_=out_sb[:, base:base + 4 * MM_FREE],
                                )
    nc.compile()
    return nc


def _get_nc():
    global _nc_cache
    if _nc_cache is None:
        _nc_cache = _build_nc()
    return _nc_cache


def _make_in_maps(x, blocks):
    # x ships as e3m4 at 2x scale; weights absorb the 1/2 (both scales are
    # powers of two, so they are exact and cancel).
    xT = np.ascontiguousarray(x.T * np.float32(2.0)).astype(
        ml_dtypes.float8_e3m4
    )
    in_maps = []
    for c in range(N_CORES):
        k0 = c * K_PER_CORE
        xTc = xT[k0:k0 + K_PER_CORE]
        xp = np.empty((8, 128, M_GROUP), ml_dtypes.float8_e3m4)
        for g in range(N_GROUPS):
            for blk in range(BLOCKS_PER_CORE):
                for mh in range(2):
                    s = g * 4 + blk * 2 + mh
                    m0 = g * M_GROUP + mh * QUAD_M
                    A = xTc[2 * blk * 128:(2 * blk + 1) * 128, m0:m0 + QUAD_M]
                    B = xTc[(2 * blk + 1) * 128:(2 * blk + 2) * 128,
                            m0:m0 + QUAD_M]
                    if s < N_SPLIT:
                        xp[s, :, 0:1024] = A[:, :1024]
                        xp[s, :, 1024:2048] = B[:, :1024]
                        xp[s, :, 2048:3072] = A[:, 1024:]
                        xp[s, :, 3072:4096] = B[:, 1024:]
                    else:
                        xp[s, :, :QUAD_M] = A
                        xp[s, :, QUAD_M:] = B
        cols = []
        for blk in range(BLOCKS_PER_CORE):
            b0 = k0 + blk * BLOCK
            w = np.tanh(blocks[b0:b0 + BLOCK, b0:b0 + BLOCK]) * np.float32(0.5)
            # [kc*128+p, ncol*128+j] -> col = kc*256 + ncol*128 + j
            cols.append(
                w.reshape(2, 128, 2, 128).transpose(1, 0, 2, 3).reshape(128, 512)
            )
        bsw = np.concatenate(cols, axis=1).astype(ml_dtypes.bfloat16)
        in_maps.append({"xP": xp, "bsw": np.ascontiguousarray(bsw)})
    return in_maps


def _run(x, blocks, **spmd_kwargs):
    res = run_bass_kernel_spmd(
        _get_nc(), _make_in_maps(x, blocks), core_ids=list(range(N_CORES)),
        **spmd_kwargs,
    )
    out = np.empty((N_ROWS, D), np.float32)
    for c in range(N_CORES):
        shard = res.results[c]["outT"]
        out[:, c * K_PER_CORE:(c + 1) * K_PER_CORE] = shard.T.astype(np.float32)
    return out, res


def kernel(x, blocks, mask=None):
    out, _ = _run(np.asarray(x), np.asarray(blocks))
    return out
